# revision 28
# baseline (speedup 1.0000x reference)
"""Tensor-parallel GQA attention layer (T=2048, dim=4096, 32 q-heads / 8 kv-heads,
D=128, interleaved RoPE, causal) for 8 Trainium2 NeuronCores.

Sharding: TP over heads. Each core owns 4 q-heads + 1 kv-head:
  - w_qkv rows (head-grouped) sharded -> per-core [768, 4096]
  - w_o columns sharded -> per-core [4096, 512]
  - x replicated
Each core computes its partial output [2048, 4096] in bf16; the host sums the
8 partials in fp32 (equivalent to the all-reduce) and casts to bf16.

Per-core schedule (PE floor ~340us):
  phase 1  qkv^T [768, 2048] accumulated i-outer/jt-inner into 6 parallel PSUM
           banks so each x-tile arrival feeds 6 matmuls; PSUM->SBUF copies on
           ACT (Copy only -- no table thrash); RoPE combine on DVE (fp32
           tables); V transposed on PE.
  phase 2  fused attention + output projection, j-chunk outer. Per (h, j):
           S^T = Kr.T @ Qr -> exp on ACT -> P^T bf16; softmax row-sums via a
           bf16 pairwise-add tree on DVE + ONE [128,128] ones-matmul
           (replaces the full ones-matmul pass, -26us PE); U^T = V.T-accum PV.
           W_o quarters interleaved between heads so ACT exp hides under
           WO/PV matmuls; output partials copied bf16 (DVE) and DMA'd per
           [128, 2048] tile.
"""
import numpy as np
import ml_dtypes

T, DIM, H, HKV, D, NCORES = 2048, 4096, 32, 8, 128, 8
HL = H // NCORES            # 4 local q heads
NJT = HL + 2                # q0..q3, k, v slabs
JL = NJT * D                # 768 local qkv rows
WO_L = HL * D               # 512 local w_o cols
NKT = DIM // D              # 32 contraction k-tiles
SCALE = float(D) ** -0.5
THETA = 10000.0
NP_BF16 = ml_dtypes.bfloat16

_CACHE = {}


def _build_nc(reps=1):
    from contextlib import ExitStack
    import concourse.bacc as bacc
    import concourse.mybir as mybir
    from concourse.tile import TileContext
    from concourse.masks import make_identity

    bf = mybir.dt.bfloat16
    f32 = mybir.dt.float32
    Exp = mybir.ActivationFunctionType.Exp

    nc = bacc.Bacc("TRN2", target_bir_lowering=False, debug=False,
                   num_devices=NCORES)
    xT_h = nc.dram_tensor("xT", [DIM, T], bf, kind="ExternalInput")
    ws_h = nc.dram_tensor("wslab", [D, NJT * NKT * D], bf, kind="ExternalInput")
    wo_h = nc.dram_tensor("woT", [WO_L, DIM], bf, kind="ExternalInput")
    cos_h = nc.dram_tensor("cosb", [D, T], f32, kind="ExternalInput")
    sin_h = nc.dram_tensor("sinb", [D, T], f32, kind="ExternalInput")
    psw_h = nc.dram_tensor("pswap", [D, D], bf, kind="ExternalInput")
    msk_h = nc.dram_tensor("mask01", [D, D], bf, kind="ExternalInput")
    out_h = nc.dram_tensor("outp", [T, DIM], bf, kind="ExternalOutput")
    xT, wsl, wo = xT_h.ap(), ws_h.ap(), wo_h.ap()
    cosb, sinb, psw, msk, outp = (cos_h.ap(), sin_h.ap(), psw_h.ap(),
                                  msk_h.ap(), out_h.ap())

    def emit_once(tc, top):
        const = top.enter_context(tc.tile_pool(name="const", bufs=1))
        pswap_sb = const.tile([D, D], bf, name="pswap_sb")
        mask_sb = const.tile([D, D], bf, name="mask_sb")
        ones_sb = const.tile([D, D], bf, name="ones_sb")
        ident_sb = const.tile([D, D], bf, name="ident_sb")
        nc.gpsimd.memset(ones_sb[:], 1.0)
        make_identity(nc, ident_sb[:])

        persist = top.enter_context(tc.tile_pool(name="persist", bufs=1))
        # Qr^T per local head + Kr^T: bf16 [128, T]
        rot_sb = [persist.tile([D, T], bf, name=f"rot{jt}", tag=f"rot{jt}")
                  for jt in range(HL + 1)]
        # V natural orientation, one [128, 128] tile per s-block
        v_sb = [persist.tile([D, D], bf, name=f"v{st}", tag=f"v{st}")
                for st in range(T // D)]

        # ---------------- phase 1: QKV + RoPE + V transpose ----------------
        with ExitStack() as ph1:
            wp = ph1.enter_context(tc.tile_pool(name="wp", bufs=1))
            xp = ph1.enter_context(tc.tile_pool(name="xp", bufs=1))
            rp = ph1.enter_context(tc.tile_pool(name="rp", bufs=1))
            qbp = ph1.enter_context(tc.tile_pool(name="qbp", bufs=3))
            swp = ph1.enter_context(tc.tile_pool(name="swp", bufs=2))
            tp = ph1.enter_context(tc.tile_pool(name="tp", bufs=4))
            qps = ph1.enter_context(tc.tile_pool(name="qps", bufs=3, space="PSUM"))
            sps = ph1.enter_context(tc.tile_pool(name="sps", bufs=2, space="PSUM"))

            cos_sb = rp.tile([D, T], f32, name="cos_sb")
            sin_sb = rp.tile([D, T], f32, name="sin_sb")

            # interleave w-slab and x DMAs in first-use order
            w_sb = [None] * NJT
            x_sb = [None] * NKT

            def dma_w(jt, chunks=1):
                wt = wp.tile([D, NKT * D], bf, name=f"w{jt}", tag=f"w{jt}")
                csz = NKT * D // chunks
                for ch in range(chunks):
                    sl = slice(ch * csz, (ch + 1) * csz)
                    nc.sync.dma_start(
                        wt[:, sl], wsl[:, jt * NKT * D:][:, sl])
                w_sb[jt] = wt
                return wt

            def dma_x(i, th, dst):
                xt = xp.tile([D, 1024], bf, name=f"x{i}", tag=f"x{i}")
                nc.sync.dma_start(xt[:], xT[i * D:(i + 1) * D,
                                             th * 1024:(th + 1) * 1024])
                dst[i] = xt

            # interleave w0/w1 chunks with the x stream in consumption order
            # so chain jt0 streams at DMA pace and w1 lands before chain jt1
            w0 = wp.tile([D, NKT * D], bf, name="w0", tag="w0")
            w1 = wp.tile([D, NKT * D], bf, name="w1", tag="w1")
            w_sb[0], w_sb[1] = w0, w1
            for i in range(NKT):
                if i % 8 == 0:
                    nc.sync.dma_start(
                        w0[:, i * D:(i + 8) * D], wsl[:, i * D:(i + 8) * D])
                if i % 8 == 4:
                    base = NKT * D
                    j0 = (i - 4) * D
                    nc.sync.dma_start(
                        w1[:, j0:j0 + 8 * D], wsl[:, base + j0:base + j0 + 8 * D])
                dma_x(i, 0, x_sb)
            dma_w(2)
            nc.sync.dma_start(pswap_sb[:], psw[:, :])
            nc.sync.dma_start(mask_sb[:], msk[:, :])
            nc.sync.dma_start(cos_sb[:], cosb[:, :])
            nc.sync.dma_start(sin_sb[:], sinb[:, :])
            for jt in range(3, NJT):
                dma_w(jt)

            def post(jt, c, ps):
                # drain PSUM for (jt, chunk c): bf16 cast on ACT, then rope/V
                qkv_bf = qbp.tile([D, 512], bf, name="qkv_bf", tag="qkv_bf")
                nc.scalar.copy(qkv_bf[:], ps[:])
                sl = slice(c * 512, (c + 1) * 512)
                if jt < 5:
                    sw_ps = sps.tile([D, 512], f32, name="sw_ps", tag="swvt")
                    nc.tensor.matmul(sw_ps[:], pswap_sb[:], qkv_bf[:],
                                     start=True, stop=True)
                    sw_bf = swp.tile([D, 512], bf, name="sw_bf", tag="sw_bf")
                    nc.scalar.copy(sw_bf[:], sw_ps[:])
                    t1 = tp.tile([D, 512], f32, name="t1", tag="t1")
                    t2t = tp.tile([D, 512], f32, name="t2t", tag="t2t")
                    nc.vector.tensor_mul(t1[:], qkv_bf[:], cos_sb[:, sl])
                    nc.vector.tensor_mul(t2t[:], sw_bf[:], sin_sb[:, sl])
                    nc.vector.tensor_add(rot_sb[jt][:, sl], t1[:], t2t[:])
                else:
                    for k8 in range(4):
                        st = c * 4 + k8
                        vt_ps = sps.tile([D, D], bf, name="vt_ps", tag="swvt")
                        nc.tensor.transpose(
                            vt_ps[:], qkv_bf[:, k8 * D:(k8 + 1) * D], ident_sb[:])
                        nc.vector.tensor_copy(v_sb[st][:], vt_ps[:])

            # jt-outer with both 512-chunks of the t-half accumulated per
            # weight tile: each [128,128] stationary load feeds 2 matmuls
            x_next = [None] * NKT
            for th in range(2):
                if th == 1:
                    x_sb[:] = x_next
                for jt in range(NJT):
                    if th == 0 and jt == 2:
                        for i in range(NKT):
                            dma_x(i, 1, x_next)
                    ps_a = qps.tile([D, 512], f32, name="ps_a", tag="psA")
                    ps_b = qps.tile([D, 512], f32, name="ps_b", tag="psB")
                    for i in range(NKT):
                        lhsT = w_sb[jt][:, i * D:(i + 1) * D]
                        nc.tensor.matmul(ps_a[:], lhsT, x_sb[i][:, 0:512],
                                         start=(i == 0), stop=(i == NKT - 1))
                        nc.tensor.matmul(ps_b[:], lhsT, x_sb[i][:, 512:1024],
                                         start=(i == 0), stop=(i == NKT - 1))
                    post(jt, 2 * th + 0, ps_a)
                    post(jt, 2 * th + 1, ps_b)

        # ---------------- phase 2: fused attention + output projection ------
        with ExitStack() as ph2:
            aop = ph2.enter_context(tc.tile_pool(name="aop", bufs=1))
            wop = ph2.enter_context(tc.tile_pool(name="wop", bufs=1))
            ptp = ph2.enter_context(tc.tile_pool(name="ptp", bufs=36))
            trp = ph2.enter_context(tc.tile_pool(name="trp", bufs=16))
            rip = ph2.enter_context(tc.tile_pool(name="rip", bufs=2))
            obp = ph2.enter_context(tc.tile_pool(name="obp", bufs=4))
            scs = ph2.enter_context(tc.tile_pool(name="scs", bufs=2, space="PSUM"))
            rps = ph2.enter_context(tc.tile_pool(name="rps", bufs=1, space="PSUM"))
            ups = ph2.enter_context(tc.tile_pool(name="ups", bufs=1, space="PSUM"))
            pos = ph2.enter_context(tc.tile_pool(name="pos", bufs=2, space="PSUM"))

            ao_sb = [aop.tile([D, T], bf, name=f"ao{h}", tag=f"ao{h}")
                     for h in range(HL)]
            wo_sb = []
            for jc in range(HL):
                wt = wop.tile([D, DIM], bf, name=f"wo{jc}", tag=f"wo{jc}")
                nc.sync.dma_start(wt[:], wo[jc * D:(jc + 1) * D, :])
                wo_sb.append(wt)

            def s_exp(h, j):
                n_st = 4 * j + 4
                pts = []
                for st in range(n_st):
                    t_off = max(0, st - 4 * j) * D
                    sc = scs.tile([D, 512], f32, name="sc", tag="sc")
                    nc.tensor.matmul(
                        sc[:, t_off:], rot_sb[HL][:, st * D:(st + 1) * D],
                        rot_sb[h][:, j * 512 + t_off:(j + 1) * 512],
                        start=True, stop=True)
                    pt = ptp.tile([D, 512], bf, name="pt", tag="pt")
                    if t_off:
                        nc.gpsimd.memset(pt[:, :t_off], 0.0)
                    nc.scalar.activation(pt[:, t_off:], sc[:, t_off:], Exp,
                                         scale=SCALE)
                    if st >= 4 * j:
                        nc.vector.tensor_mul(pt[:, t_off:t_off + D],
                                             pt[:, t_off:t_off + D], mask_sb[:])
                    pts.append(pt)
                return pts

            def pv(h, j, pts):
                n_st = len(pts)
                # bf16 pairwise tree -> single [128, 512] tile of partial
                # sums; first level on the otherwise-idle Pool engine
                cur = list(pts)
                level = 0
                while len(cur) > 1:
                    nxt = []
                    for a in range(0, len(cur) - 1, 2):
                        tr = trp.tile([D, 512], bf, name="tr", tag="tr")
                        eng = nc.gpsimd if (level == 0 and a % 4 == 0) else nc.vector
                        eng.tensor_add(tr[:], cur[a][:], cur[a + 1][:])
                        nxt.append(tr)
                    if len(cur) % 2:
                        nxt.append(cur[-1])
                    cur = nxt
                    level += 1
                # PV chain first -- the ones-matmul waits on the add tree, so
                # emitting it after PV keeps PE busy while the tree finishes
                u_ps = ups.tile([D, 512], f32, name="u_ps", tag="u")
                for st in range(n_st):
                    t_off = max(0, st - 4 * j) * D
                    nc.tensor.matmul(u_ps[:, t_off:], v_sb[st][:],
                                     pts[st][:, t_off:],
                                     start=(st == 0), stop=(st == n_st - 1))
                r_ps = rps.tile([D, 512], f32, name="r_ps", tag="r")
                nc.tensor.matmul(r_ps[:], ones_sb[:], cur[0][:],
                                 start=True, stop=True)
                r_inv = rip.tile([D, 512], f32, name="r_inv", tag="ri")
                nc.vector.reciprocal(r_inv[:], r_ps[:])
                nc.vector.tensor_mul(ao_sb[h][:, j * 512:(j + 1) * 512],
                                     u_ps[:], r_inv[:])

            def wo_quarter(tt):
                # jc-outer / mb-inner: each stationary ao-tile load feeds 2
                # matmuls; single po per q keeps cross-q pipelining
                for half2 in range(2):
                    ob = obp.tile([D, 2048], bf, name="ob", tag="ob")
                    for q in range(2):
                        po = pos.tile([D, 1024], f32, name="po", tag="po")
                        base = half2 * 2048 + q * 1024
                        for jc in range(HL):
                            lhsT = ao_sb[jc][:, tt * D:(tt + 1) * D]
                            for mb in range(2):
                                nc.tensor.matmul(
                                    po[:, mb * 512:(mb + 1) * 512],
                                    lhsT,
                                    wo_sb[jc][:, base + mb * 512:base + (mb + 1) * 512],
                                    start=(jc == 0), stop=(jc == HL - 1))
                        nc.vector.tensor_copy(ob[:, q * 1024:(q + 1) * 1024],
                                              po[:])
                    nc.sync.dma_start(
                        outp[tt * D:(tt + 1) * D,
                             half2 * 2048:(half2 + 1) * 2048], ob[:])

            for j in range(4):
                for h in range(HL):
                    pts = s_exp(h, j)
                    if j > 0:
                        wo_quarter((j - 1) * 4 + h)
                    pv(h, j, pts)
            for h in range(HL):
                wo_quarter(12 + h)

    from contextlib import ExitStack as _ES
    with TileContext(nc) as tc:
        for _rep in range(reps):
            with _ES() as top:
                emit_once(tc, top)

    nc.compile()
    return nc


def get_nc(reps=1):
    key = ("nc", reps)
    if key not in _CACHE:
        _CACHE[key] = _build_nc(reps)
    return _CACHE[key]


def host_prep(x, w_qkv, w_o):
    """Returns per-core input maps (numpy)."""
    x = np.asarray(x)
    w_qkv = np.asarray(w_qkv)
    w_o = np.asarray(w_o)
    xT = np.ascontiguousarray(x.T)
    inv_freq = 1.0 / (THETA ** (np.arange(0, D, 2, dtype=np.float64) / D))
    ang = np.arange(T, dtype=np.float64)[:, None] * inv_freq[None, :]
    cosb = np.empty((D, T), np.float32)
    sinb = np.empty((D, T), np.float32)
    cosb[0::2] = np.cos(ang).T
    cosb[1::2] = np.cos(ang).T
    sinb[0::2] = -np.sin(ang).T
    sinb[1::2] = np.sin(ang).T
    pswap = np.zeros((D, D), NP_BF16)
    for d in range(D):
        pswap[d, d ^ 1] = 1
    mask01 = np.triu(np.ones((D, D), np.float32)).astype(NP_BF16)
    in_maps = []
    for c in range(NCORES):
        wq_rows = w_qkv[c * HL * D:(c + 1) * HL * D]
        wk_rows = w_qkv[H * D + c * D: H * D + (c + 1) * D]
        wv_rows = w_qkv[(H + HKV) * D + c * D:(H + HKV) * D + (c + 1) * D]
        w_c = np.concatenate([wq_rows, wk_rows, wv_rows], axis=0)  # [768, 4096]
        w_cT = np.ascontiguousarray(w_c.T)                         # [4096, 768]
        # slab[p, ((jt*32)+i)*128 + cc] = w_cT[i*128+p, jt*128+cc]
        wslab = np.ascontiguousarray(
            w_cT.reshape(NKT, D, NJT, D).transpose(1, 2, 0, 3)
            .reshape(D, NJT * NKT * D))
        in_maps.append({
            "xT": xT,
            "wslab": wslab,
            "woT": np.ascontiguousarray(w_o[:, c * WO_L:(c + 1) * WO_L].T),
            "cosb": cosb, "sinb": sinb, "pswap": pswap, "mask01": mask01,
        })
    return in_maps


def kernel(x, w_qkv, w_o):
    from concourse.bass_utils import run_bass_kernel_spmd
    nc = get_nc()
    in_maps = host_prep(x, w_qkv, w_o)
    res = run_bass_kernel_spmd(nc, in_maps, list(range(NCORES)))
    acc = np.zeros((T, DIM), np.float32)
    for c in range(NCORES):
        acc += res.results[c]["outp"].astype(np.float32)
    return acc.astype(NP_BF16)
